# revision 34
# baseline (speedup 1.0000x reference)
"""Trainium2 Bass kernel for windowed (banded) self-attention MLP block.

Reference computation (per batch b):
    h = relu(x @ W1 + b1)                      # [S, H]
    q = h @ Wq                                 # [S, H]
    scores[s, w] = q[s] . h_pad[s + w] / 32    # window w in [0, 33), h zero-padded by A=16
    wgt = softmax(scores, axis=w)
    out[s] = sum_w wgt[s, w] * h_pad[s + w]

Sharding: 8 cores, each takes 1024 consecutive tokens of the flattened
[B*S] = 8192 token stream (2 cores per batch element; shards never cross a
batch boundary).  Each core redundantly computes h for a 16-token halo on
each side, so no cross-core communication is needed.

Per-core DRAM layouts (host prepares):
    xa  [128, 4, 1152] bf16   x^T chunked along IN
    w1  [128, 8, 4, 128] bf16 W1 chunks, [p, hc, c, j] = W1[c*128+p, hc*128+j]
    wq  [128, 8, 1024] bf16   (Wq / 32) chunked along H_in
    b1c [128, 8] f32          b1 as per-hc bias columns
    hm  [128, 2] f32          halo validity multipliers (left, right)
    out [1024, 1024] bf16     (host casts back to f32)

On-chip stages (bf16 matmul operands, fp32 PSUM accumulation):
    A:  hT[hc, t] = relu(W1^T @ xT + b1)   H-on-partitions, 1056 tokens,
        bias+relu fused in one DVE tensor_scalar; halo cols zeroed via hm
    T:  hh[t, h]  = PE-transpose of hT, 9 token blocks x 8 hc, copies
        PSUM->SBUF split across DVE/ACT (interleaved with A so copies
        drain behind the next A tile's matmuls)
    B:  qT[ho, t] = (Wq/32)^T @ hT  in 8 chunks of 128 tokens, chunk i
        feeding stage-D tile i-1 (software pipeline: D's softmax chain
        hides behind the next B chunk's matmuls)
    D:  per 128-token tile: scores psum pre-initialized with the band
        mask, then scores += qT^T @ hT_window  [128, 160]
        p = exp(scores) (bf16) + denominator via ACT accum_out,
        pT via PE transpose; out = (pT^T @ hh_window) * (1/den)
"""

import sys

import numpy as np

try:
    import concourse.bass as bass
except ImportError:
    sys.path.insert(0, "/opt/trn_rl_repo")
    import concourse.bass as bass

import ml_dtypes

import concourse.mybir as mybir
import concourse.tile as tile
from concourse import bacc
from concourse.bass_utils import run_bass_kernel_spmd

BF16 = ml_dtypes.bfloat16

B, S, IN, H = 4, 2048, 512, 1024
A = 16
WND = 2 * A + 1            # 33 window positions
NCORES = 8
TOK = (B * S) // NCORES    # 1024 tokens per core
TOKH = TOK + 2 * A         # 1056 with halo
TOKP = 9 * 128             # 1152 zero-padded token slots
NT = TOK // 128            # 8 output tiles per core
WIN = 128 + 2 * A          # 160-token window per 128-token tile
NEG = -30000.0             # additive mask for out-of-band positions

f32 = mybir.dt.float32
bf16 = mybir.dt.bfloat16
AF = mybir.ActivationFunctionType
ALU = mybir.AluOpType


def _band_mask():
    """[128, WIN] additive mask: row t allows window cols t..t+32."""
    m = np.full((128, WIN), NEG, dtype=np.float32)
    for t in range(128):
        m[t, t : t + WND] = 0.0
    return m


def _kernel_body(tc, nc, xa_d, w1_d, wq_d, b1c_d, hm_d, out_d, mask_d, id_d):
    with (
        tc.tile_pool(name="const", bufs=1) as cpool,
        tc.tile_pool(name="wts", bufs=1) as wpool,
        tc.tile_pool(name="acts", bufs=1) as apool,
    ):
        xa = wpool.tile([128, 4, TOKP], bf16, tag="xa")
        w1 = wpool.tile([128, 8, 4, 128], bf16, tag="w1")
        wq = wpool.tile([128, 8, H], bf16, tag="wq")
        b1c = cpool.tile([128, 8], f32, tag="b1c")
        hm = cpool.tile([128, 2], f32, tag="hm")
        mask_sb = cpool.tile([128, WIN], bf16, tag="mask")
        id_sb = cpool.tile([128, 128], bf16, tag="ident")
        warm = wpool.tile([128, 256], bf16, tag="warm")
        nc.vector.memset(warm[:], 0.0)

        # DMA issue costs ~0.63us of sequencer occupancy each, so split the
        # input loads across the two HWDGE queues (Sync + Scalar), ordered by
        # first use.  Stage A runs hc-major so it can start streaming as soon
        # as the first xa chunk and one 128KB w1 chunk have landed; later w1
        # chunks (one per hc) each arrive well inside the previous hc's
        # compute time.
        # All bulk loads go on the sync queue in strict need-order: the 16
        # SDMA rings drain FIFO, so a big early transfer of late-needed data
        # (wq) would stall the stage-A xa/w1 stream behind it.
        nc.sync.dma_start(xa[:, :, 0:128], xa_d[:, :, 0:128])
        nc.sync.dma_start(w1[:, 0], w1_d[:, 0])
        nc.sync.dma_start(xa[:, :, 128:640], xa_d[:, :, 128:640])
        nc.sync.dma_start(xa[:, :, 640:TOKH], xa_d[:, :, 640:TOKH])
        for hc in range(1, 8):
            nc.sync.dma_start(w1[:, hc], w1_d[:, hc])
        nc.sync.dma_start(wq[:, 0:4], wq_d[:, 0:4])
        nc.sync.dma_start(wq[:, 4:8], wq_d[:, 4:8])

        nc.scalar.dma_start(hm[:], hm_d[:])
        nc.scalar.dma_start(b1c[:], b1c_d[:])
        nc.scalar.dma_start(id_sb[:], id_d[:])
        nc.scalar.dma_start(mask_sb[:], mask_d[:])

        hT = apool.tile([128, 8, TOKH], bf16, tag="hT")
        hh = apool.tile([128, 9, H], bf16, tag="hh")
        qT = apool.tile([128, 8, TOK], bf16, tag="qT")

        with (
            tc.tile_pool(name="dtmp", bufs=2) as dpool,
            tc.tile_pool(name="outp", bufs=3) as opool,
        ):
            with tc.tile_pool(name="psA", bufs=1, space="PSUM") as psA:
                # Short PE warm-up on a zeroed scratch tile: releases the PE
                # clock gate without delaying the first real matmul much.
                for _ in range(8):
                    wps = psA.tile([128, 128], f32, tag="warm", bufs=1)
                    nc.tensor.matmul(
                        wps[:], warm[:, 0:128], warm[:, 0:128],
                        start=True, stop=True,
                    )

                # A: hT = relu(W1^T @ xT + b1); hc-major so compute starts
                # after one w1 chunk.  bias+relu fused on DVE:
                # (psum + b1) max 0 -> bf16
                A_TILES = ((0, 128), (128, 640), (640, TOKH))
                for hc in range(8):
                    for t0, t1 in A_TILES:
                        sl = slice(t0, t1)
                        ps = psA.tile([128, t1 - t0], f32, tag="pa", bufs=4)
                        for c in range(4):
                            nc.tensor.matmul(
                                ps[:],
                                w1[:, hc, c, :],
                                xa[:, c, sl],
                                start=(c == 0),
                                stop=(c == 3),
                            )
                        nc.vector.tensor_scalar(
                            hT[:, hc, sl],
                            ps[:],
                            b1c[:, hc : hc + 1],
                            0.0,
                            ALU.add,
                            ALU.max,
                        )
                        # zero halo cols that fall outside this core's batch,
                        # right after the tile that produced them (so the
                        # first transpose block isn't gated on the last tile)
                        if t0 == 0:
                            nc.vector.tensor_scalar_mul(
                                hT[:, hc, 0:A], hT[:, hc, 0:A], hm[:, 0:1]
                            )
                        elif t1 == TOKH:
                            nc.vector.tensor_scalar_mul(
                                hT[:, hc, TOK + A : TOKH],
                                hT[:, hc, TOK + A : TOKH],
                                hm[:, 1:2],
                            )

            with tc.tile_pool(name="psBD", bufs=1, space="PSUM") as psBD:
                # T: hh[t, h] = hT^T via PE transposes; 4 transposes batched
                # per PSUM tile so each PSUM->SBUF copy moves 512 cols.
                # Transpose pairs interleave with B chunks so the DVE/ACT
                # copies drain behind B matmuls instead of stalling the PE.
                def t_block(tb):
                    ncols = 32 if tb == 8 else 128
                    for hg in range(2):
                        # shares the "pav" tag: transposes finish before the
                        # first d_tile needs a pav accumulator
                        pt = psBD.tile([128, 512], bf16, tag="pav", bufs=3)
                        for k in range(4):
                            nc.tensor.transpose(
                                pt[0:ncols, k * 128 : (k + 1) * 128],
                                hT[:, hg * 4 + k, tb * 128 : tb * 128 + ncols],
                                id_sb[:],
                            )
                        dst = hh[0:ncols, tb, hg * 512 : (hg + 1) * 512]
                        if (2 * tb + hg) % 2:
                            nc.vector.tensor_copy(dst, pt[0:ncols, :])
                        else:
                            nc.scalar.copy(dst, pt[0:ncols, :])

                # B: qT = (Wq/32)^T @ hT in 8 chunks of 128 tokens,
                # interleaved with stage-D tiles so D's softmax chain hides
                # behind B matmuls.
                def b_chunk(i):
                    # two PSUM half-chunks (1 bank each) per 128-token chunk
                    for hb in range(2):
                        ps = psBD.tile([128, 4, 128], f32, tag="pb", bufs=2)
                        for ho4 in range(4):
                            ho = hb * 4 + ho4
                            for hi in range(8):
                                nc.tensor.matmul(
                                    ps[:, ho4, :],
                                    wq[:, hi, ho * 128 : (ho + 1) * 128],
                                    hT[:, hi, A + i * 128 : A + (i + 1) * 128],
                                    start=(hi == 0),
                                    stop=(hi == 7),
                                )
                        dst = qT[:, hb * 4 : (hb + 1) * 4, i * 128 : (i + 1) * 128]
                        if (2 * i + hb) % 2:
                            nc.vector.tensor_copy(dst, ps[:])
                        else:
                            nc.scalar.copy(dst, ps[:])

                def d_tile(T):
                    # the band mask enters PSUM via an identity matmul that
                    # opens the accumulation group: ps_s[t,w] = mask[t,w],
                    # then the 8 score matmuls accumulate on top
                    ps_s = psBD.tile([128, WIN], f32, tag="ps", bufs=2)
                    nc.tensor.matmul(
                        ps_s[:], id_sb[:], mask_sb[:], start=True, stop=False
                    )
                    for hc in range(8):
                        nc.tensor.matmul(
                            ps_s[:],
                            qT[:, hc, T * 128 : (T + 1) * 128],
                            hT[:, hc, T * 128 : T * 128 + WIN],
                            start=False,
                            stop=(hc == 7),
                        )
                    p_sb = dpool.tile([128, WIN], bf16, tag="p")
                    den = dpool.tile([128, 1], f32, tag="den")
                    nc.scalar.activation(p_sb[:], ps_s[:], AF.Exp, accum_out=den[:])
                    rcp = dpool.tile([128, 1], f32, tag="rcp")
                    nc.vector.reciprocal(rcp[:], den[:])

                    ptm = psBD.tile([128, 256], bf16, tag="ptm", bufs=1)
                    nc.tensor.transpose(ptm[:, 0:128], p_sb[:, 0:128], id_sb[:])
                    nc.tensor.transpose(
                        ptm[0:32, 128:256], p_sb[:, 128:WIN], id_sb[:]
                    )
                    pta_sb = dpool.tile([128, 256], bf16, tag="pta")
                    nc.vector.tensor_copy(pta_sb[:, 0:128], ptm[:, 0:128])
                    nc.vector.tensor_copy(pta_sb[0:32, 128:256], ptm[0:32, 128:256])

                    out_sb = opool.tile([128, H], bf16, tag="osb")
                    tsl = slice(T * 128, (T + 1) * 128)
                    for half in range(2):
                        hsl = slice(half * 512, (half + 1) * 512)
                        pav = psBD.tile([128, 512], f32, tag="pav", bufs=3)
                        nc.tensor.matmul(
                            pav[:], pta_sb[:, 0:128], hh[:, T, hsl],
                            start=True, stop=False,
                        )
                        nc.tensor.matmul(
                            pav[:], pta_sb[0:32, 128:256], hh[0:32, T + 1, hsl],
                            start=False, stop=True,
                        )
                        if T == NT - 1:
                            # final tile: quarter-split the normalize across
                            # both engines and both DMA queues to shorten the
                            # drain after the last matmul
                            q0 = slice(half * 512, half * 512 + 256)
                            q1 = slice(half * 512 + 256, (half + 1) * 512)
                            nc.vector.tensor_scalar_mul(
                                out_sb[:, q0], pav[:, 0:256], rcp[:]
                            )
                            nc.scalar.mul(out_sb[:, q1], pav[:, 256:512], rcp[:])
                            nc.sync.dma_start(out_d[tsl, q0], out_sb[:, q0])
                            nc.scalar.dma_start(out_d[tsl, q1], out_sb[:, q1])
                        elif half == 0:
                            # alternate the normalize+copyback between DVE and
                            # ACT so the two halves run on different engines
                            nc.vector.tensor_scalar_mul(
                                out_sb[:, hsl], pav[:], rcp[:]
                            )
                            nc.sync.dma_start(out_d[tsl, hsl], out_sb[:, hsl])
                        else:
                            nc.scalar.mul(out_sb[:, hsl], pav[:], rcp[:])
                            nc.sync.dma_start(out_d[tsl, hsl], out_sb[:, hsl])

                # All transpose blocks run right after stage A: they need no
                # new data, so they fill the PE while the wq load (needed by
                # B) finishes.  Then B chunks interleave with D tiles so D's
                # softmax chain hides behind B matmuls.
                for tb in range(9):
                    t_block(tb)
                b_chunk(0)
                b_chunk(1)
                for T in range(NT):
                    if T + 2 < NT:
                        b_chunk(T + 2)
                    d_tile(T)


def build_nc():
    nc = bacc.Bacc("TRN2", target_bir_lowering=False, debug=False, num_devices=NCORES)
    xa_d = nc.dram_tensor("xa", [128, 4, TOKP], bf16, kind="ExternalInput")
    w1_d = nc.dram_tensor("w1", [128, 8, 4, 128], bf16, kind="ExternalInput")
    wq_d = nc.dram_tensor("wq", [128, 8, H], bf16, kind="ExternalInput")
    b1c_d = nc.dram_tensor("b1c", [128, 8], f32, kind="ExternalInput")
    hm_d = nc.dram_tensor("hm", [128, 2], f32, kind="ExternalInput")
    out_d = nc.dram_tensor("out", [TOK, H], bf16, kind="ExternalOutput")
    mask_d = nc.inline_tensor(_band_mask().astype(BF16), "bandmask")
    id_d = nc.inline_tensor(np.eye(128, dtype=BF16), "ident")

    with tile.TileContext(nc) as tc:
        _kernel_body(tc, nc, xa_d, w1_d, wq_d, b1c_d, hm_d, out_d, mask_d, id_d)
    nc.compile()
    return nc


def make_inputs(x, W1, b1, Wq):
    """Host-side shard prep (numpy only; not part of HW time)."""
    x = np.asarray(x, dtype=np.float32)
    W1 = np.asarray(W1, dtype=np.float32)
    b1 = np.asarray(b1, dtype=np.float32)
    Wq = np.asarray(Wq, dtype=np.float32)

    # w1a[p, hc, c, j] = W1[c*128 + p, hc*128 + j]
    w1a = np.zeros((128, 8, 4, 128), dtype=BF16)
    for hc in range(8):
        for c in range(4):
            w1a[:, hc, c, :] = W1[
                c * 128 : (c + 1) * 128, hc * 128 : (hc + 1) * 128
            ].astype(BF16)

    b1c = np.ascontiguousarray(b1.reshape(8, 128).T).astype(np.float32)  # [128, 8]

    wqs = (Wq / np.sqrt(np.float32(H))).astype(BF16)
    wqa = np.zeros((128, 8, H), dtype=BF16)
    for c in range(8):
        wqa[:, c, :] = wqs[c * 128 : (c + 1) * 128, :]

    in_maps = []
    for core in range(NCORES):
        b, half = divmod(core, 2)
        lo = half * TOK - A
        hi = half * TOK + TOK + A
        s0, s1 = max(lo, 0), min(hi, S)
        xs = np.zeros((TOKH, IN), dtype=np.float32)
        xs[s0 - lo : s1 - lo] = x[b, s0:s1]
        xT = np.ascontiguousarray(xs.T).astype(BF16)  # [512, 1056]
        xa = np.zeros((128, 4, TOKP), dtype=BF16)
        for c in range(4):
            xa[:, c, :TOKH] = xT[c * 128 : (c + 1) * 128, :]
        hmv = np.zeros((128, 2), dtype=np.float32)
        hmv[:, 0] = 1.0 if lo >= 0 else 0.0
        hmv[:, 1] = 1.0 if hi <= S else 0.0
        in_maps.append({"xa": xa, "w1": w1a, "wq": wqa, "b1c": b1c, "hm": hmv})
    return in_maps


_NC_CACHE = {}


def get_nc():
    if "nc" not in _NC_CACHE:
        _NC_CACHE["nc"] = build_nc()
    return _NC_CACHE["nc"]


def kernel(x, W1, b1, Wq, atten_size, _trace=False, _trace_kwargs=None):
    assert int(atten_size) == A, f"kernel hardcodes atten_size=16, got {atten_size}"
    nc = get_nc()
    in_maps = make_inputs(x, W1, b1, Wq)
    kw = {}
    if _trace:
        kw = dict(trace=True, trace_kwargs=_trace_kwargs or {})
    res = run_bass_kernel_spmd(nc, in_maps, core_ids=list(range(NCORES)), **kw)
    out = np.stack([r["out"].astype(np.float32) for r in res.results])
    out = out.reshape(B, S, H)
    if _trace:
        return out, res
    return out


if __name__ == "__main__":
    import jax

    key = jax.random.key(0)
    k1, k2, k3, k4 = jax.random.split(key, 4)
    x = np.asarray(jax.random.normal(k1, (B, S, IN), dtype=np.float32))
    W1 = np.asarray(
        jax.random.normal(k2, (IN, H), dtype=np.float32) * (1.0 / np.sqrt(IN))
    )
    b1 = np.asarray(jax.random.normal(k3, (H,), dtype=np.float32) * 0.02)
    Wq = np.asarray(
        jax.random.normal(k4, (H, H), dtype=np.float32) * (1.0 / np.sqrt(H))
    )
    out = kernel(x, W1, b1, Wq, 16)
    print("out", out.shape, out.dtype, float(np.abs(out).max()))


# revision 45
# speedup vs baseline: 1.1085x; 1.1085x over previous
"""Trainium2 Bass kernel for windowed (banded) self-attention MLP block.

Reference computation (per batch b):
    h = relu(x @ W1 + b1)                      # [S, H]
    q = h @ Wq                                 # [S, H]
    scores[s, w] = q[s] . h_pad[s + w] / 32    # window w in [0, 33), h zero-padded by A=16
    wgt = softmax(scores, axis=w)
    out[s] = sum_w wgt[s, w] * h_pad[s + w]

Sharding: 8 cores, each takes 1024 consecutive tokens of the flattened
[B*S] = 8192 token stream (2 cores per batch element; shards never cross a
batch boundary).  Each core redundantly computes h for a 16-token halo on
each side, so no cross-core communication is needed.

Per-core DRAM layouts (host prepares):
    xa  [128, 4, 1152] bf16   x^T chunked along IN
    w1  [128, 8, 4, 128] bf16 W1 chunks, [p, hc, c, j] = W1[c*128+p, hc*128+j]
    wq  [128, 8, 1024] bf16   (Wq / 32) chunked along H_in
    b1c [128, 8] f32          b1 as per-hc bias columns
    hm  [128, 2] f32          halo validity multipliers (left, right)
    out [1024, 1024] bf16     (host casts back to f32)

On-chip stages (bf16 matmul operands, fp32 PSUM accumulation):
    A:  hT[hc, t] = relu(W1^T @ xT + b1)   H-on-partitions, 1056 tokens,
        bias+relu fused in one DVE tensor_scalar; halo cols zeroed via hm
    T:  hh[t, h]  = PE-transpose of hT, 9 token blocks x 8 hc, copies
        PSUM->SBUF split across DVE/ACT (interleaved with A so copies
        drain behind the next A tile's matmuls)
    B:  qT[ho, t] = (Wq/32)^T @ hT  in 8 chunks of 128 tokens, chunk i
        feeding stage-D tile i-1 (software pipeline: D's softmax chain
        hides behind the next B chunk's matmuls)
    D:  per 128-token tile: scores psum pre-initialized with the band
        mask, then scores += qT^T @ hT_window  [128, 160]
        p = exp(scores) (bf16) + denominator via ACT accum_out,
        pT via PE transpose; out = (pT^T @ hh_window) * (1/den)
"""

import sys

import numpy as np

try:
    import concourse.bass as bass
except ImportError:
    sys.path.insert(0, "/opt/trn_rl_repo")
    import concourse.bass as bass

import ml_dtypes

import concourse.mybir as mybir
import concourse.tile as tile
from concourse import bacc
from concourse.bass_utils import run_bass_kernel_spmd

BF16 = ml_dtypes.bfloat16

B, S, IN, H = 4, 2048, 512, 1024
A = 16
WND = 2 * A + 1            # 33 window positions
NCORES = 8
TOK = (B * S) // NCORES    # 1024 tokens per core
TOKH = TOK + 2 * A         # 1056 with halo
TOKP = 9 * 128             # 1152 zero-padded token slots
NT = TOK // 128            # 8 output tiles per core
WIN = 128 + 2 * A          # 160-token window per 128-token tile
NEG = -30000.0             # additive mask for out-of-band positions

f32 = mybir.dt.float32
bf16 = mybir.dt.bfloat16
f8 = mybir.dt.float8e4
F8 = ml_dtypes.float8_e4m3fn
DR = mybir.MatmulPerfMode.DoubleRow
WQS = 2048.0               # fp8 scale on (Wq/32), applied on host
HSC = 16.0                 # fp8 scale on h, applied in the on-chip cast
AF = mybir.ActivationFunctionType
ALU = mybir.AluOpType


def _band_mask():
    """[128, WIN] additive mask: row t allows window cols t..t+32."""
    m = np.full((128, WIN), NEG, dtype=np.float32)
    for t in range(128):
        m[t, t : t + WND] = 0.0
    return m


def _kernel_body(tc, nc, xa_d, w1_d, wq_d, b1c_d, hm_d, out_d, mask_d, id_d):
    with (
        tc.tile_pool(name="const", bufs=1) as cpool,
        tc.tile_pool(name="wts", bufs=1) as wpool,
        tc.tile_pool(name="acts", bufs=1) as apool,
    ):
        xa = wpool.tile([128, 4, TOKP], bf16, tag="xa")
        w1 = wpool.tile([128, 8, 4, 128], bf16, tag="w1")
        wq = wpool.tile([128, 4, 2, H], f8, tag="wq")
        b1c = cpool.tile([128, 8], f32, tag="b1c")
        hm = cpool.tile([128, 2], f32, tag="hm")
        mask_sb = cpool.tile([128, WIN], bf16, tag="mask")
        id_sb = cpool.tile([128, 128], bf16, tag="ident")
        warm = wpool.tile([128, 256], bf16, tag="warm")
        nc.vector.memset(warm[:], 0.0)

        # DMA issue costs ~0.63us of sequencer occupancy each, so split the
        # input loads across the two HWDGE queues (Sync + Scalar), ordered by
        # first use.  Stage A runs hc-major so it can start streaming as soon
        # as the first xa chunk and one 128KB w1 chunk have landed; later w1
        # chunks (one per hc) each arrive well inside the previous hc's
        # compute time.
        # All bulk loads go on the sync queue in strict need-order: the 16
        # SDMA rings drain FIFO, so a big early transfer of late-needed data
        # (wq) would stall the stage-A xa/w1 stream behind it.
        nc.sync.dma_start(xa[:, :, 0:128], xa_d[:, :, 0:128])
        nc.sync.dma_start(w1[:, 0], w1_d[:, 0])
        nc.sync.dma_start(xa[:, :, 128:640], xa_d[:, :, 128:640])
        nc.sync.dma_start(xa[:, :, 640:TOKH], xa_d[:, :, 640:TOKH])
        for hc in range(1, 8):
            nc.sync.dma_start(w1[:, hc], w1_d[:, hc])
        nc.sync.dma_start(wq[:, 0:2], wq_d[:, 0:2])
        nc.sync.dma_start(wq[:, 2:4], wq_d[:, 2:4])

        nc.scalar.dma_start(hm[:], hm_d[:])
        nc.scalar.dma_start(b1c[:], b1c_d[:])
        nc.scalar.dma_start(id_sb[:], id_d[:])
        nc.scalar.dma_start(mask_sb[:], mask_d[:])

        hT = apool.tile([128, 8, TOKH], bf16, tag="hT")
        hT8 = apool.tile([128, 4, 2, TOK], f8, tag="hT8")
        hh = apool.tile([128, 9, H], bf16, tag="hh")
        qT = apool.tile([128, 8, TOK], bf16, tag="qT")

        with (
            tc.tile_pool(name="dtmp", bufs=2) as dpool,
            tc.tile_pool(name="outp", bufs=3) as opool,
        ):
            with tc.tile_pool(name="psA", bufs=1, space="PSUM") as psA:
                # Short PE warm-up on a zeroed scratch tile: releases the PE
                # clock gate without delaying the first real matmul much.
                for _ in range(8):
                    wps = psA.tile([128, 128], f32, tag="warm", bufs=1)
                    nc.tensor.matmul(
                        wps[:], warm[:, 0:128], warm[:, 0:128],
                        start=True, stop=True,
                    )

                # A: hT = relu(W1^T @ xT + b1); hc-major so compute starts
                # after one w1 chunk.  bias+relu fused on DVE:
                # (psum + b1) max 0 -> bf16
                A_TILES = ((0, 128), (128, 640), (640, TOKH))
                for hc in range(8):
                    for t0, t1 in A_TILES:
                        sl = slice(t0, t1)
                        ps = psA.tile([128, t1 - t0], f32, tag="pa", bufs=4)
                        for c in range(4):
                            nc.tensor.matmul(
                                ps[:],
                                w1[:, hc, c, :],
                                xa[:, c, sl],
                                start=(c == 0),
                                stop=(c == 3),
                            )
                        nc.vector.tensor_scalar(
                            hT[:, hc, sl],
                            ps[:],
                            b1c[:, hc : hc + 1],
                            0.0,
                            ALU.add,
                            ALU.max,
                        )
                        # zero halo cols that fall outside this core's batch,
                        # right after the tile that produced them (so the
                        # first transpose block isn't gated on the last tile)
                        if t0 == 0:
                            nc.vector.tensor_scalar_mul(
                                hT[:, hc, 0:A], hT[:, hc, 0:A], hm[:, 0:1]
                            )
                        elif t1 == TOKH:
                            nc.vector.tensor_scalar_mul(
                                hT[:, hc, TOK + A : TOKH],
                                hT[:, hc, TOK + A : TOKH],
                                hm[:, 1:2],
                            )
                    # scaled fp8 copy of this hc's core tokens for the
                    # DoubleRow stage-B matmuls (halo cols are outside the
                    # copied range, so no ordering hazard with the halo fix)
                    dst8 = hT8[:, hc // 2, hc % 2, :]
                    if hc % 2:
                        nc.vector.tensor_scalar_mul(
                            dst8, hT[:, hc, A : A + TOK], HSC
                        )
                    else:
                        nc.scalar.mul(dst8, hT[:, hc, A : A + TOK], HSC)

            with tc.tile_pool(name="psBD", bufs=1, space="PSUM") as psBD:
                # T: hh[t, h] = hT^T via PE transposes; 4 transposes batched
                # per PSUM tile so each PSUM->SBUF copy moves 512 cols.
                # Transpose pairs interleave with B chunks so the DVE/ACT
                # copies drain behind B matmuls instead of stalling the PE.
                def t_block(tb):
                    ncols = 32 if tb == 8 else 128
                    for hg in range(2):
                        # shares the "pav" tag: transposes finish before the
                        # first d_tile needs a pav accumulator
                        pt = psBD.tile([128, 512], bf16, tag="pav", bufs=3)
                        for k in range(4):
                            nc.tensor.transpose(
                                pt[0:ncols, k * 128 : (k + 1) * 128],
                                hT[:, hg * 4 + k, tb * 128 : tb * 128 + ncols],
                                id_sb[:],
                            )
                        dst = hh[0:ncols, tb, hg * 512 : (hg + 1) * 512]
                        if (2 * tb + hg) % 2:
                            nc.vector.tensor_copy(dst, pt[0:ncols, :])
                        else:
                            nc.scalar.copy(dst, pt[0:ncols, :])

                # B: qT = (Wq/32)^T @ hT as fp8 DoubleRow matmuls (each
                # contracts 256 H dims: 2 fp8 values packed per partition).
                # Wide token ranges keep the free dim >= 256 so the 2x-long
                # LDWEIGHTS stays hidden; the 1/(WQS*HSC) unscale folds into
                # the PSUM->qT copy.
                def b_range(t0, t1):
                    w = t1 - t0
                    for ho in range(8):
                        ps = psBD.tile([128, 512], f32, tag="pb", bufs=2)
                        for g in range(4):
                            nc.tensor.matmul(
                                ps[:, 0:w],
                                wq[:, g, :, ho * 128 : (ho + 1) * 128],
                                hT8[:, g, :, t0:t1],
                                start=(g == 0),
                                stop=(g == 3),
                                perf_mode=DR,
                            )
                        dst = qT[:, ho, t0:t1]
                        if ho % 2:
                            nc.vector.tensor_scalar_mul(
                                dst, ps[:, 0:w], 1.0 / (WQS * HSC)
                            )
                        else:
                            nc.scalar.mul(dst, ps[:, 0:w], 1.0 / (WQS * HSC))

                def d_tile(T):
                    # the band mask enters PSUM via an identity matmul that
                    # opens the accumulation group: ps_s[t,w] = mask[t,w],
                    # then the 8 score matmuls accumulate on top
                    ps_s = psBD.tile([128, WIN], f32, tag="ps", bufs=2)
                    nc.tensor.matmul(
                        ps_s[:], id_sb[:], mask_sb[:], start=True, stop=False
                    )
                    for hc in range(8):
                        nc.tensor.matmul(
                            ps_s[:],
                            qT[:, hc, T * 128 : (T + 1) * 128],
                            hT[:, hc, T * 128 : T * 128 + WIN],
                            start=False,
                            stop=(hc == 7),
                        )
                    p_sb = dpool.tile([128, WIN], bf16, tag="p")
                    den = dpool.tile([128, 1], f32, tag="den")
                    nc.scalar.activation(p_sb[:], ps_s[:], AF.Exp, accum_out=den[:])
                    rcp = dpool.tile([128, 1], f32, tag="rcp")
                    nc.vector.reciprocal(rcp[:], den[:])

                    ptm = psBD.tile([128, 256], bf16, tag="ptm", bufs=1)
                    nc.tensor.transpose(ptm[:, 0:128], p_sb[:, 0:128], id_sb[:])
                    nc.tensor.transpose(
                        ptm[0:32, 128:256], p_sb[:, 128:WIN], id_sb[:]
                    )
                    pta_sb = dpool.tile([128, 256], bf16, tag="pta")
                    nc.vector.tensor_copy(pta_sb[:, 0:128], ptm[:, 0:128])
                    nc.vector.tensor_copy(pta_sb[0:32, 128:256], ptm[0:32, 128:256])

                    out_sb = opool.tile([128, H], bf16, tag="osb")
                    tsl = slice(T * 128, (T + 1) * 128)
                    for half in range(2):
                        hsl = slice(half * 512, (half + 1) * 512)
                        pav = psBD.tile([128, 512], f32, tag="pav", bufs=3)
                        nc.tensor.matmul(
                            pav[:], pta_sb[:, 0:128], hh[:, T, hsl],
                            start=True, stop=False,
                        )
                        nc.tensor.matmul(
                            pav[:], pta_sb[0:32, 128:256], hh[0:32, T + 1, hsl],
                            start=False, stop=True,
                        )
                        if T == NT - 1:
                            # final tile: quarter-split the normalize across
                            # both engines and both DMA queues to shorten the
                            # drain after the last matmul
                            q0 = slice(half * 512, half * 512 + 256)
                            q1 = slice(half * 512 + 256, (half + 1) * 512)
                            nc.vector.tensor_scalar_mul(
                                out_sb[:, q0], pav[:, 0:256], rcp[:]
                            )
                            nc.scalar.mul(out_sb[:, q1], pav[:, 256:512], rcp[:])
                            nc.sync.dma_start(out_d[tsl, q0], out_sb[:, q0])
                            nc.scalar.dma_start(out_d[tsl, q1], out_sb[:, q1])
                        elif half == 0:
                            # alternate the normalize+copyback between DVE and
                            # ACT so the two halves run on different engines
                            nc.vector.tensor_scalar_mul(
                                out_sb[:, hsl], pav[:], rcp[:]
                            )
                            nc.sync.dma_start(out_d[tsl, hsl], out_sb[:, hsl])
                        else:
                            nc.scalar.mul(out_sb[:, hsl], pav[:], rcp[:])
                            nc.sync.dma_start(out_d[tsl, hsl], out_sb[:, hsl])

                # All transpose blocks run right after stage A: they need no
                # new data, so they fill the PE while the wq load (needed by
                # B) finishes.  Then B chunks interleave with D tiles so D's
                # softmax chain hides behind B matmuls.
                for tb in range(9):
                    t_block(tb)
                # B token ranges sized so each D tile's qT is ready just in
                # time; later D tiles run back-to-back once B is done
                b_range(0, 512)
                d_tile(0)
                b_range(512, 768)
                d_tile(1)
                b_range(768, 896)
                d_tile(2)
                b_range(896, 1024)
                for T in range(3, NT):
                    d_tile(T)


def build_nc():
    nc = bacc.Bacc("TRN2", target_bir_lowering=False, debug=False, num_devices=NCORES)
    xa_d = nc.dram_tensor("xa", [128, 4, TOKP], bf16, kind="ExternalInput")
    w1_d = nc.dram_tensor("w1", [128, 8, 4, 128], bf16, kind="ExternalInput")
    wq_d = nc.dram_tensor("wq", [128, 4, 2, H], f8, kind="ExternalInput")
    b1c_d = nc.dram_tensor("b1c", [128, 8], f32, kind="ExternalInput")
    hm_d = nc.dram_tensor("hm", [128, 2], f32, kind="ExternalInput")
    out_d = nc.dram_tensor("out", [TOK, H], bf16, kind="ExternalOutput")
    mask_d = nc.inline_tensor(_band_mask().astype(BF16), "bandmask")
    id_d = nc.inline_tensor(np.eye(128, dtype=BF16), "ident")

    with tile.TileContext(nc) as tc:
        _kernel_body(tc, nc, xa_d, w1_d, wq_d, b1c_d, hm_d, out_d, mask_d, id_d)
    nc.compile()
    return nc


def make_inputs(x, W1, b1, Wq):
    """Host-side shard prep (numpy only; not part of HW time)."""
    x = np.asarray(x, dtype=np.float32)
    W1 = np.asarray(W1, dtype=np.float32)
    b1 = np.asarray(b1, dtype=np.float32)
    Wq = np.asarray(Wq, dtype=np.float32)

    # w1a[p, hc, c, j] = W1[c*128 + p, hc*128 + j]
    w1a = np.zeros((128, 8, 4, 128), dtype=BF16)
    for hc in range(8):
        for c in range(4):
            w1a[:, hc, c, :] = W1[
                c * 128 : (c + 1) * 128, hc * 128 : (hc + 1) * 128
            ].astype(BF16)

    b1c = np.ascontiguousarray(b1.reshape(8, 128).T).astype(np.float32)  # [128, 8]

    # wqa[p, g, kt, n] = (Wq/32)[(2g+kt)*128 + p, n] * WQS in fp8 e4m3;
    # stage B contracts (g, kt, p) pairs against the matching hT8 layout
    wqs = Wq / np.sqrt(np.float32(H)) * np.float32(WQS)
    wqa = np.zeros((128, 4, 2, H), dtype=F8)
    for g in range(4):
        for kt in range(2):
            wqa[:, g, kt, :] = wqs[
                (2 * g + kt) * 128 : (2 * g + kt + 1) * 128, :
            ].astype(F8)

    in_maps = []
    for core in range(NCORES):
        b, half = divmod(core, 2)
        lo = half * TOK - A
        hi = half * TOK + TOK + A
        s0, s1 = max(lo, 0), min(hi, S)
        xs = np.zeros((TOKH, IN), dtype=np.float32)
        xs[s0 - lo : s1 - lo] = x[b, s0:s1]
        xT = np.ascontiguousarray(xs.T).astype(BF16)  # [512, 1056]
        xa = np.zeros((128, 4, TOKP), dtype=BF16)
        for c in range(4):
            xa[:, c, :TOKH] = xT[c * 128 : (c + 1) * 128, :]
        hmv = np.zeros((128, 2), dtype=np.float32)
        hmv[:, 0] = 1.0 if lo >= 0 else 0.0
        hmv[:, 1] = 1.0 if hi <= S else 0.0
        in_maps.append({"xa": xa, "w1": w1a, "wq": wqa, "b1c": b1c, "hm": hmv})
    return in_maps


_NC_CACHE = {}


def get_nc():
    if "nc" not in _NC_CACHE:
        _NC_CACHE["nc"] = build_nc()
    return _NC_CACHE["nc"]


def kernel(x, W1, b1, Wq, atten_size, _trace=False, _trace_kwargs=None):
    assert int(atten_size) == A, f"kernel hardcodes atten_size=16, got {atten_size}"
    nc = get_nc()
    in_maps = make_inputs(x, W1, b1, Wq)
    kw = {}
    if _trace:
        kw = dict(trace=True, trace_kwargs=_trace_kwargs or {})
    res = run_bass_kernel_spmd(nc, in_maps, core_ids=list(range(NCORES)), **kw)
    out = np.stack([r["out"].astype(np.float32) for r in res.results])
    out = out.reshape(B, S, H)
    if _trace:
        return out, res
    return out


if __name__ == "__main__":
    import jax

    key = jax.random.key(0)
    k1, k2, k3, k4 = jax.random.split(key, 4)
    x = np.asarray(jax.random.normal(k1, (B, S, IN), dtype=np.float32))
    W1 = np.asarray(
        jax.random.normal(k2, (IN, H), dtype=np.float32) * (1.0 / np.sqrt(IN))
    )
    b1 = np.asarray(jax.random.normal(k3, (H,), dtype=np.float32) * 0.02)
    Wq = np.asarray(
        jax.random.normal(k4, (H, H), dtype=np.float32) * (1.0 / np.sqrt(H))
    )
    out = kernel(x, W1, b1, Wq, 16)
    print("out", out.shape, out.dtype, float(np.abs(out).max()))


# revision 46
# speedup vs baseline: 1.1241x; 1.0141x over previous
"""Trainium2 Bass kernel for windowed (banded) self-attention MLP block.

Reference computation (per batch b):
    h = relu(x @ W1 + b1)                      # [S, H]
    q = h @ Wq                                 # [S, H]
    scores[s, w] = q[s] . h_pad[s + w] / 32    # window w in [0, 33), h zero-padded by A=16
    wgt = softmax(scores, axis=w)
    out[s] = sum_w wgt[s, w] * h_pad[s + w]

Sharding: 8 cores, each takes 1024 consecutive tokens of the flattened
[B*S] = 8192 token stream (2 cores per batch element; shards never cross a
batch boundary).  Each core redundantly computes h for a 16-token halo on
each side, so no cross-core communication is needed.

Per-core DRAM layouts (host prepares):
    xa  [128, 4, 1152] bf16   x^T chunked along IN
    w1  [128, 8, 4, 128] bf16 W1 chunks, [p, hc, c, j] = W1[c*128+p, hc*128+j]
    wq  [128, 8, 1024] bf16   (Wq / 32) chunked along H_in
    b1c [128, 8] f32          b1 as per-hc bias columns
    hm  [128, 2] f32          halo validity multipliers (left, right)
    out [1024, 1024] bf16     (host casts back to f32)

On-chip stages (bf16 matmul operands, fp32 PSUM accumulation):
    A:  hT[hc, t] = relu(W1^T @ xT + b1)   H-on-partitions, 1056 tokens,
        bias+relu fused in one DVE tensor_scalar; halo cols zeroed via hm
    T:  hh[t, h]  = PE-transpose of hT, 9 token blocks x 8 hc, copies
        PSUM->SBUF split across DVE/ACT (interleaved with A so copies
        drain behind the next A tile's matmuls)
    B:  qT[ho, t] = (Wq/32)^T @ hT  in 8 chunks of 128 tokens, chunk i
        feeding stage-D tile i-1 (software pipeline: D's softmax chain
        hides behind the next B chunk's matmuls)
    D:  per 128-token tile: scores psum pre-initialized with the band
        mask, then scores += qT^T @ hT_window  [128, 160]
        p = exp(scores) (bf16) + denominator via ACT accum_out,
        pT via PE transpose; out = (pT^T @ hh_window) * (1/den)
"""

import sys

import numpy as np

try:
    import concourse.bass as bass
except ImportError:
    sys.path.insert(0, "/opt/trn_rl_repo")
    import concourse.bass as bass

import ml_dtypes

import concourse.mybir as mybir
import concourse.tile as tile
from concourse import bacc
from concourse.bass_utils import run_bass_kernel_spmd

BF16 = ml_dtypes.bfloat16

B, S, IN, H = 4, 2048, 512, 1024
A = 16
WND = 2 * A + 1            # 33 window positions
NCORES = 8
TOK = (B * S) // NCORES    # 1024 tokens per core
TOKH = TOK + 2 * A         # 1056 with halo
TOKP = 9 * 128             # 1152 zero-padded token slots
NT = TOK // 128            # 8 output tiles per core
WIN = 128 + 2 * A          # 160-token window per 128-token tile
NEG = -30000.0             # additive mask for out-of-band positions

f32 = mybir.dt.float32
bf16 = mybir.dt.bfloat16
AF = mybir.ActivationFunctionType
ALU = mybir.AluOpType


def _band_mask():
    """[128, WIN] additive mask: row t allows window cols t..t+32."""
    m = np.full((128, WIN), NEG, dtype=np.float32)
    for t in range(128):
        m[t, t : t + WND] = 0.0
    return m


def _kernel_body(tc, nc, xa_d, w1_d, wq_d, b1c_d, hm_d, out_d, mask_d, id_d):
    with (
        tc.tile_pool(name="const", bufs=1) as cpool,
        tc.tile_pool(name="wts", bufs=1) as wpool,
        tc.tile_pool(name="acts", bufs=1) as apool,
    ):
        xa = wpool.tile([128, 4, TOKP], bf16, tag="xa")
        w1 = wpool.tile([128, 8, 4, 128], bf16, tag="w1")
        wq = wpool.tile([128, 8, H], bf16, tag="wq")
        b1c = cpool.tile([128, 8], f32, tag="b1c")
        hm = cpool.tile([128, 2], f32, tag="hm")
        mask_sb = cpool.tile([128, WIN], bf16, tag="mask")
        id_sb = cpool.tile([128, 128], bf16, tag="ident")
        warm = wpool.tile([128, 256], bf16, tag="warm")
        nc.vector.memset(warm[:], 0.0)

        # DMA issue costs ~0.63us of sequencer occupancy each, so split the
        # input loads across the two HWDGE queues (Sync + Scalar), ordered by
        # first use.  Stage A runs hc-major so it can start streaming as soon
        # as the first xa chunk and one 128KB w1 chunk have landed; later w1
        # chunks (one per hc) each arrive well inside the previous hc's
        # compute time.
        # All bulk loads go on the sync queue in strict need-order: the 16
        # SDMA rings drain FIFO, so a big early transfer of late-needed data
        # (wq) would stall the stage-A xa/w1 stream behind it.
        nc.sync.dma_start(xa[:, :, 0:128], xa_d[:, :, 0:128])
        nc.sync.dma_start(w1[:, 0], w1_d[:, 0])
        nc.sync.dma_start(xa[:, :, 128:640], xa_d[:, :, 128:640])
        nc.sync.dma_start(xa[:, :, 640:TOKH], xa_d[:, :, 640:TOKH])
        for hc in range(1, 8):
            nc.sync.dma_start(w1[:, hc], w1_d[:, hc])
        nc.sync.dma_start(wq[:, 0:4], wq_d[:, 0:4])
        nc.sync.dma_start(wq[:, 4:8], wq_d[:, 4:8])

        nc.scalar.dma_start(hm[:], hm_d[:])
        nc.scalar.dma_start(b1c[:], b1c_d[:])
        nc.scalar.dma_start(id_sb[:], id_d[:])
        nc.scalar.dma_start(mask_sb[:], mask_d[:])

        hT = apool.tile([128, 8, TOKH], bf16, tag="hT")
        hh = apool.tile([128, 9, H], bf16, tag="hh")
        qT = apool.tile([128, 8, TOK], bf16, tag="qT")

        with (
            tc.tile_pool(name="dtmp", bufs=2) as dpool,
            tc.tile_pool(name="outp", bufs=3) as opool,
        ):
            with tc.tile_pool(name="psA", bufs=1, space="PSUM") as psA:
                # Short PE warm-up on a zeroed scratch tile: releases the PE
                # clock gate without delaying the first real matmul much.
                for _ in range(8):
                    wps = psA.tile([128, 128], f32, tag="warm", bufs=1)
                    nc.tensor.matmul(
                        wps[:], warm[:, 0:128], warm[:, 0:128],
                        start=True, stop=True,
                    )

                # A: hT = relu(W1^T @ xT + b1); hc-major so compute starts
                # after one w1 chunk.  bias+relu fused on DVE:
                # (psum + b1) max 0 -> bf16
                A_TILES = ((0, 128), (128, 640), (640, TOKH))
                for hc in range(8):
                    for t0, t1 in A_TILES:
                        sl = slice(t0, t1)
                        ps = psA.tile([128, t1 - t0], f32, tag="pa", bufs=4)
                        for c in range(4):
                            nc.tensor.matmul(
                                ps[:],
                                w1[:, hc, c, :],
                                xa[:, c, sl],
                                start=(c == 0),
                                stop=(c == 3),
                            )
                        nc.vector.tensor_scalar(
                            hT[:, hc, sl],
                            ps[:],
                            b1c[:, hc : hc + 1],
                            0.0,
                            ALU.add,
                            ALU.max,
                        )
                        # zero halo cols that fall outside this core's batch,
                        # right after the tile that produced them (so the
                        # first transpose block isn't gated on the last tile)
                        if t0 == 0:
                            nc.vector.tensor_scalar_mul(
                                hT[:, hc, 0:A], hT[:, hc, 0:A], hm[:, 0:1]
                            )
                        elif t1 == TOKH:
                            nc.vector.tensor_scalar_mul(
                                hT[:, hc, TOK + A : TOKH],
                                hT[:, hc, TOK + A : TOKH],
                                hm[:, 1:2],
                            )

            with tc.tile_pool(name="psBD", bufs=1, space="PSUM") as psBD:
                # T: hh[t, h] = hT^T via PE transposes; 4 transposes batched
                # per PSUM tile so each PSUM->SBUF copy moves 512 cols.
                # Transpose pairs interleave with B chunks so the DVE/ACT
                # copies drain behind B matmuls instead of stalling the PE.
                def t_block(tb):
                    ncols = 32 if tb == 8 else 128
                    for hg in range(2):
                        # shares the "pav" tag: transposes finish before the
                        # first d_tile needs a pav accumulator
                        pt = psBD.tile([128, 512], bf16, tag="pav", bufs=3)
                        for k in range(4):
                            nc.tensor.transpose(
                                pt[0:ncols, k * 128 : (k + 1) * 128],
                                hT[:, hg * 4 + k, tb * 128 : tb * 128 + ncols],
                                id_sb[:],
                            )
                        dst = hh[0:ncols, tb, hg * 512 : (hg + 1) * 512]
                        if (2 * tb + hg) % 2:
                            nc.vector.tensor_copy(dst, pt[0:ncols, :])
                        else:
                            nc.scalar.copy(dst, pt[0:ncols, :])

                # B: qT = (Wq/32)^T @ hT in 8 chunks of 128 tokens,
                # interleaved with stage-D tiles so D's softmax chain hides
                # behind B matmuls.
                def b_chunk(i):
                    # two PSUM half-chunks (1 bank each) per 128-token chunk
                    for hb in range(2):
                        ps = psBD.tile([128, 4, 128], f32, tag="pb", bufs=2)
                        for ho4 in range(4):
                            ho = hb * 4 + ho4
                            for hi in range(8):
                                nc.tensor.matmul(
                                    ps[:, ho4, :],
                                    wq[:, hi, ho * 128 : (ho + 1) * 128],
                                    hT[:, hi, A + i * 128 : A + (i + 1) * 128],
                                    start=(hi == 0),
                                    stop=(hi == 7),
                                )
                        dst = qT[:, hb * 4 : (hb + 1) * 4, i * 128 : (i + 1) * 128]
                        if (2 * i + hb) % 2:
                            nc.vector.tensor_copy(dst, ps[:])
                        else:
                            nc.scalar.copy(dst, ps[:])

                def d_tile(T):
                    # the band mask enters PSUM via an identity matmul that
                    # opens the accumulation group: ps_s[t,w] = mask[t,w],
                    # then the 8 score matmuls accumulate on top
                    ps_s = psBD.tile([128, WIN], f32, tag="ps", bufs=2)
                    nc.tensor.matmul(
                        ps_s[:], id_sb[:], mask_sb[:], start=True, stop=False
                    )
                    for hc in range(8):
                        nc.tensor.matmul(
                            ps_s[:],
                            qT[:, hc, T * 128 : (T + 1) * 128],
                            hT[:, hc, T * 128 : T * 128 + WIN],
                            start=False,
                            stop=(hc == 7),
                        )
                    p_sb = dpool.tile([128, WIN], bf16, tag="p")
                    den = dpool.tile([128, 1], f32, tag="den")
                    nc.scalar.activation(p_sb[:], ps_s[:], AF.Exp, accum_out=den[:])
                    rcp = dpool.tile([128, 1], f32, tag="rcp")
                    nc.vector.reciprocal(rcp[:], den[:])

                    ptm = psBD.tile([128, 256], bf16, tag="ptm", bufs=1)
                    nc.tensor.transpose(ptm[:, 0:128], p_sb[:, 0:128], id_sb[:])
                    nc.tensor.transpose(
                        ptm[0:32, 128:256], p_sb[:, 128:WIN], id_sb[:]
                    )
                    pta_sb = dpool.tile([128, 256], bf16, tag="pta")
                    nc.vector.tensor_copy(pta_sb[:, 0:128], ptm[:, 0:128])
                    nc.vector.tensor_copy(pta_sb[0:32, 128:256], ptm[0:32, 128:256])

                    out_sb = opool.tile([128, H], bf16, tag="osb")
                    tsl = slice(T * 128, (T + 1) * 128)
                    for half in range(2):
                        hsl = slice(half * 512, (half + 1) * 512)
                        pav = psBD.tile([128, 512], f32, tag="pav", bufs=3)
                        nc.tensor.matmul(
                            pav[:], pta_sb[:, 0:128], hh[:, T, hsl],
                            start=True, stop=False,
                        )
                        nc.tensor.matmul(
                            pav[:], pta_sb[0:32, 128:256], hh[0:32, T + 1, hsl],
                            start=False, stop=True,
                        )
                        if T == NT - 1:
                            # final tile: quarter-split the normalize across
                            # both engines and both DMA queues to shorten the
                            # drain after the last matmul
                            q0 = slice(half * 512, half * 512 + 256)
                            q1 = slice(half * 512 + 256, (half + 1) * 512)
                            nc.vector.tensor_scalar_mul(
                                out_sb[:, q0], pav[:, 0:256], rcp[:]
                            )
                            nc.scalar.mul(out_sb[:, q1], pav[:, 256:512], rcp[:])
                            nc.sync.dma_start(out_d[tsl, q0], out_sb[:, q0])
                            nc.scalar.dma_start(out_d[tsl, q1], out_sb[:, q1])
                        elif half == 0:
                            # alternate the normalize+copyback between DVE and
                            # ACT so the two halves run on different engines
                            nc.vector.tensor_scalar_mul(
                                out_sb[:, hsl], pav[:], rcp[:]
                            )
                            nc.sync.dma_start(out_d[tsl, hsl], out_sb[:, hsl])
                        else:
                            nc.scalar.mul(out_sb[:, hsl], pav[:], rcp[:])
                            nc.sync.dma_start(out_d[tsl, hsl], out_sb[:, hsl])

                # All transpose blocks run right after stage A: they need no
                # new data, so they fill the PE while the wq load (needed by
                # B) finishes.  Then B chunks interleave with D tiles so D's
                # softmax chain hides behind B matmuls.
                for tb in range(9):
                    t_block(tb)
                b_chunk(0)
                b_chunk(1)
                for T in range(NT):
                    if T + 2 < NT:
                        b_chunk(T + 2)
                    d_tile(T)


def build_nc():
    nc = bacc.Bacc("TRN2", target_bir_lowering=False, debug=False, num_devices=NCORES)
    xa_d = nc.dram_tensor("xa", [128, 4, TOKP], bf16, kind="ExternalInput")
    w1_d = nc.dram_tensor("w1", [128, 8, 4, 128], bf16, kind="ExternalInput")
    wq_d = nc.dram_tensor("wq", [128, 8, H], bf16, kind="ExternalInput")
    b1c_d = nc.dram_tensor("b1c", [128, 8], f32, kind="ExternalInput")
    hm_d = nc.dram_tensor("hm", [128, 2], f32, kind="ExternalInput")
    out_d = nc.dram_tensor("out", [TOK, H], bf16, kind="ExternalOutput")
    mask_d = nc.inline_tensor(_band_mask().astype(BF16), "bandmask")
    id_d = nc.inline_tensor(np.eye(128, dtype=BF16), "ident")

    with tile.TileContext(nc) as tc:
        _kernel_body(tc, nc, xa_d, w1_d, wq_d, b1c_d, hm_d, out_d, mask_d, id_d)
    nc.compile()
    return nc


def make_inputs(x, W1, b1, Wq):
    """Host-side shard prep (numpy only; not part of HW time)."""
    x = np.asarray(x, dtype=np.float32)
    W1 = np.asarray(W1, dtype=np.float32)
    b1 = np.asarray(b1, dtype=np.float32)
    Wq = np.asarray(Wq, dtype=np.float32)

    # w1a[p, hc, c, j] = W1[c*128 + p, hc*128 + j]
    w1a = np.zeros((128, 8, 4, 128), dtype=BF16)
    for hc in range(8):
        for c in range(4):
            w1a[:, hc, c, :] = W1[
                c * 128 : (c + 1) * 128, hc * 128 : (hc + 1) * 128
            ].astype(BF16)

    b1c = np.ascontiguousarray(b1.reshape(8, 128).T).astype(np.float32)  # [128, 8]

    wqs = (Wq / np.sqrt(np.float32(H))).astype(BF16)
    wqa = np.zeros((128, 8, H), dtype=BF16)
    for c in range(8):
        wqa[:, c, :] = wqs[c * 128 : (c + 1) * 128, :]

    in_maps = []
    for core in range(NCORES):
        b, half = divmod(core, 2)
        lo = half * TOK - A
        hi = half * TOK + TOK + A
        s0, s1 = max(lo, 0), min(hi, S)
        xs = np.zeros((TOKH, IN), dtype=np.float32)
        xs[s0 - lo : s1 - lo] = x[b, s0:s1]
        xT = np.ascontiguousarray(xs.T).astype(BF16)  # [512, 1056]
        xa = np.zeros((128, 4, TOKP), dtype=BF16)
        for c in range(4):
            xa[:, c, :TOKH] = xT[c * 128 : (c + 1) * 128, :]
        hmv = np.zeros((128, 2), dtype=np.float32)
        hmv[:, 0] = 1.0 if lo >= 0 else 0.0
        hmv[:, 1] = 1.0 if hi <= S else 0.0
        in_maps.append({"xa": xa, "w1": w1a, "wq": wqa, "b1c": b1c, "hm": hmv})
    return in_maps


_NC_CACHE = {}


def get_nc():
    if "nc" not in _NC_CACHE:
        _NC_CACHE["nc"] = build_nc()
    return _NC_CACHE["nc"]


def kernel(x, W1, b1, Wq, atten_size, _trace=False, _trace_kwargs=None):
    assert int(atten_size) == A, f"kernel hardcodes atten_size=16, got {atten_size}"
    nc = get_nc()
    in_maps = make_inputs(x, W1, b1, Wq)
    kw = {}
    if _trace:
        kw = dict(trace=True, trace_kwargs=_trace_kwargs or {})
    res = run_bass_kernel_spmd(nc, in_maps, core_ids=list(range(NCORES)), **kw)
    out = np.stack([r["out"].astype(np.float32) for r in res.results])
    out = out.reshape(B, S, H)
    if _trace:
        return out, res
    return out


if __name__ == "__main__":
    import jax

    key = jax.random.key(0)
    k1, k2, k3, k4 = jax.random.split(key, 4)
    x = np.asarray(jax.random.normal(k1, (B, S, IN), dtype=np.float32))
    W1 = np.asarray(
        jax.random.normal(k2, (IN, H), dtype=np.float32) * (1.0 / np.sqrt(IN))
    )
    b1 = np.asarray(jax.random.normal(k3, (H,), dtype=np.float32) * 0.02)
    Wq = np.asarray(
        jax.random.normal(k4, (H, H), dtype=np.float32) * (1.0 / np.sqrt(H))
    )
    out = kernel(x, W1, b1, Wq, 16)
    print("out", out.shape, out.dtype, float(np.abs(out).max()))
